# revision 34
# baseline (speedup 1.0000x reference)
"""DeepFM forward kernel for 8 Trainium2 NeuronCores (Bass/Tile).

Math (per batch row b):
    lin[b] = x[b] @ w
    C[b]   = sum_k (x[b] @ v)_k^2
    B[b]   = sum_f s[f] * x[b,f]^2,   s[f] = sum_k v[f,k]^2
    out[b] = sigmoid(lin[b] + b0 + 0.5*C[b] - 0.5*B[b])

Data-parallel: batch 16384 sharded 8 ways (2048 rows/core); parameters
replicated.

Key reformulation: ship u = x * sqrt(s) (per-feature scale folded on host)
in fp16, with v' = v/sqrt(s), w' = w/sqrt(s) as the stationary matrix.
Then xv = u @ v', lin = u @ w', and B = sum_f u_f^2 — the only on-chip
elementwise op is an unscaled square. fp16 halves HBM traffic and runs
the PE at full rate.

Schedule notes (cost-model driven; ~29.3us/core vs 81.2us baseline):
  - u streams on BOTH HWDGE queues (SP: even stripes + quartered stripes
    0/15; ACT: odd stripes + 14) — transfers from different queues
    overlap in the DMA fabric, ~2x effective rate; stream ends ~14us.
  - B routing: stripes {0,12,13,14,15} feed ones-matmuls into PSUM on
    the PE; stripes 1..11 accumulate u^2 into four fp16 chain
    accumulators (DVE adds), combined pairwise and folded with two
    4-matmul sets mid-stream.
  - Squares split across DVE (1127ns) and GPS (1707ns); late stripes
    (10..15) split half/half so neither queue-end sticks out.
  - Constants materialize via DVE memsets; only scalar b rides a DMA.
  - ACT runs only Copy/Sigmoid; a warmup Sigmoid makes the single
    (hoisted) act-table load pick a set covering both, so no table load
    ever lands on the critical path.
  - Tail: all remaining A-matmuls (stripes 12-15) issue before any late
    B-matmul so psumA stops early; per chunk: copy [xv;lin]->fp16 (ACT),
    square-in-place (DVE/GPS), 4 B-matmuls + C-matmul (PE), Sigmoid,
    y DMA — chunk-pipelined.
"""

import numpy as np

import concourse.bass as bass
import concourse.tile as tile
from concourse import bacc, mybir
from concourse.bass_utils import run_bass_kernel_spmd

BATCH, FIELD, EMBED = 16384, 2048, 64
NCORES = 8
BS = BATCH // NCORES   # 2048 batch rows per core
NCHUNK = 512           # psum free-dim per matmul
KTILES = FIELD // 128  # 16 contraction stripes
NCHUNKS = BS // NCHUNK  # 4 batch chunks per core
M = EMBED + 1          # 65 stationary columns: v' plus w'

F32 = mybir.dt.float32
F16 = mybir.dt.float16
AF = mybir.ActivationFunctionType

# B-accumulation chains (value = chain id 0..3); seeds are the first
# member of each chain (its sq writes the accumulator directly).
CHAINS = [[1, 2, 3], [4, 5, 6], [7, 8, 9], [10, 11]]
PE_B = {0, 12, 13, 14, 15}
# Engine for each stripe's square: v=DVE, g=GPS(pool), s=split DVE+GPS
SQ_ENG = {1: "g", 2: "v", 3: "g", 4: "g", 5: "v", 6: "g", 7: "g",
          8: "g", 9: "v", 10: "s", 11: "s", 12: "s", 13: "s", 14: "s"}


def _build_nc():
    nc = bacc.Bacc("TRN2", target_bir_lowering=False, debug=False)

    # stripe-major u: partition p, col k*BS + b  <->  u[k*128+p, b]
    ut = nc.declare_dram_parameter("ut", [128, KTILES * BS], F16, isOutput=False)
    vwi = nc.declare_dram_parameter("vwi", [128, KTILES * M], F16, isOutput=False)
    bvec = nc.declare_dram_parameter("bvec", [1, 1], F32, isOutput=False)
    y = nc.declare_dram_parameter("y", [NCHUNKS, NCHUNK], F32, isOutput=True)

    with tile.TileContext(nc) as tc:
        with (
            tc.tile_pool(name="consts", bufs=1) as consts,
            tc.tile_pool(name="uin", bufs=8) as uin,
            tc.tile_pool(name="uq", bufs=1) as uq,
            tc.tile_pool(name="sqp", bufs=8) as sqp,
            tc.tile_pool(name="accs", bufs=1) as accs,
            tc.tile_pool(name="redrhs", bufs=4) as redrhs,
            tc.tile_pool(name="outp", bufs=2) as outp,
            tc.tile_pool(name="psA", bufs=NCHUNKS, space="PSUM") as psA,
            tc.tile_pool(name="psB", bufs=NCHUNKS, space="PSUM") as psB,
        ):
            vw = consts.tile([128, KTILES * M], F16)
            nc.scalar.dma_start(vw[:, :], vwi[:, :])
            b_sb = consts.tile([1, 1], F32)
            onesn_sb = consts.tile([128, 1], F16)
            nc.vector.memset(onesn_sb[:, :], -0.5)
            red_sb = consts.tile([M, 1], F16)
            nc.vector.memset(red_sb[0:EMBED, :], 0.5)
            nc.vector.memset(red_sb[EMBED:M, :], 1.0)

            psumA = [
                psA.tile([M, NCHUNK], F32, name=f"psumA{n}", tag="psumA")
                for n in range(NCHUNKS)
            ]
            psumB = [
                psB.tile([1, NCHUNK], F32, name=f"psumB{n}", tag="psumB")
                for n in range(NCHUNKS)
            ]

            acc = [accs.tile([128, BS], F16, name=f"acc{i}") for i in range(4)]
            chain_of = {k: ci for ci, ch in enumerate(CHAINS) for k in ch}
            seeds = {ch[0] for ch in CHAINS}

            utiles = {}

            ACT_STRIPES = {1, 3, 5, 7, 9, 11, 12}

            def load(k):
                if k in (13, 14):
                    eng = nc.gpsimd
                    t = uin.tile([128, BS], F16, name=f"u{k}", tag="u")
                    eng.dma_start(t[:, :], ut[:, k * BS:(k + 1) * BS])
                    utiles[k] = t
                    return
                eng = nc.scalar if k in ACT_STRIPES else nc.sync
                t = uin.tile([128, BS], F16, name=f"u{k}", tag="u")
                eng.dma_start(t[:, :], ut[:, k * BS:(k + 1) * BS])
                utiles[k] = t

            first_b = [True] * NCHUNKS

            def bmm(n, src_cols):
                nc.tensor.matmul(
                    psumB[n][:, :], onesn_sb[:, :], src_cols,
                    start=first_b[n], stop=False,
                )
                first_b[n] = False

            def process(k):
                vw_k = vw[:, k * M:(k + 1) * M]
                u_k = utiles[k]
                if k in seeds:
                    sq_k = acc[chain_of[k]]
                else:
                    sq_k = sqp.tile([128, BS], F16, name=f"sq{k}", tag="sq")
                for n in range(NCHUNKS):
                    sl = slice(n * NCHUNK, (n + 1) * NCHUNK)
                    nc.tensor.matmul(
                        psumA[n][:, :], vw_k, u_k[:, sl],
                        start=False, stop=False,
                    )
                eng = SQ_ENG[k]
                if eng == "v":
                    nc.vector.tensor_mul(sq_k[:, :], u_k[:, :], u_k[:, :])
                elif eng == "g":
                    nc.gpsimd.tensor_mul(sq_k[:, :], u_k[:, :], u_k[:, :])
                else:
                    h = BS // 2
                    nc.vector.tensor_mul(sq_k[:, :h], u_k[:, :h], u_k[:, :h])
                    nc.gpsimd.tensor_mul(sq_k[:, h:], u_k[:, h:], u_k[:, h:])
                if k in PE_B:
                    for n in range(NCHUNKS):
                        sl = slice(n * NCHUNK, (n + 1) * NCHUNK)
                        bmm(n, sq_k[:, sl])
                elif k not in seeds:
                    a = acc[chain_of[k]]
                    nc.vector.tensor_add(a[:, :], a[:, :], sq_k[:, :])

            # stripe 0 on SP, quartered so the PE (and GPS/DVE) start early
            u0 = uq.tile([128, BS], F16, name="uqt0", tag="uq0")
            sq0 = sqp.tile([128, BS], F16, name="sq0", tag="sq")
            vw_0 = vw[:, 0:M]
            for n in range(NCHUNKS):
                sl = slice(n * NCHUNK, (n + 1) * NCHUNK)
                nc.sync.dma_start(u0[:, sl], ut[:, n * NCHUNK:(n + 1) * NCHUNK])
                nc.tensor.matmul(
                    psumA[n][:, :], vw_0, u0[:, sl], start=True, stop=False,
                )
                eng0 = nc.gpsimd if n < 2 else nc.vector
                eng0.tensor_mul(sq0[:, sl], u0[:, sl], u0[:, sl])
                bmm(n, sq0[:, sl])
            for k in range(1, KTILES - 1):
                load(k)
            warm = consts.tile([1, 1], F16)
            nc.scalar.activation(warm[:, :], onesn_sb[0:1, 0:1], AF.Sigmoid)

            for k in range(1, 7):
                process(k)
            # c1 += c2 (chunk-sliced so later folds stagger per chunk)
            for n in range(NCHUNKS):
                sl = slice(n * NCHUNK, (n + 1) * NCHUNK)
                nc.vector.tensor_add(acc[0][:, sl], acc[0][:, sl], acc[1][:, sl])
            for k in range(7, 10):
                process(k)
            # fold c12 as soon as it is complete
            for n in range(NCHUNKS):
                sl = slice(n * NCHUNK, (n + 1) * NCHUNK)
                bmm(n, acc[0][:, sl])
            process(10)
            process(11)
            # c3 += c4 on GPS (chunk-sliced), then fold it
            for n in range(NCHUNKS):
                sl = slice(n * NCHUNK, (n + 1) * NCHUNK)
                nc.vector.tensor_add(acc[2][:, sl], acc[2][:, sl], acc[3][:, sl])
            for n in range(NCHUNKS):
                sl = slice(n * NCHUNK, (n + 1) * NCHUNK)
                bmm(n, acc[2][:, sl])
            # ---- tail: A-matmuls first (unblock psumA stops), then late
            # B-matmuls, then the chunk-pipelined epilogue ----
            kL = KTILES - 1
            uL = uq.tile([128, BS], F16, name="uqt15", tag="uq15")
            sqL = sqp.tile([128, BS], F16, name="sq15", tag="sq")
            vw_L = vw[:, kL * M:(kL + 1) * M]
            sqtail = {}
            rhstiles = {}
            for k in range(12, 15):
                vw_k = vw[:, k * M:(k + 1) * M]
                u_k = utiles[k]
                for n in range(NCHUNKS):
                    sl = slice(n * NCHUNK, (n + 1) * NCHUNK)
                    nc.tensor.matmul(
                        psumA[n][:, :], vw_k, u_k[:, sl],
                        start=False, stop=False,
                    )
                sqtail[k] = sqp.tile([128, BS], F16, name=f"sq{k}", tag="sq")
            # chunk-sliced squares so each chunk's B-matmuls unblock as
            # soon as its own slices land; stripe 12 rides ACT's idle
            # window (Square is in the loaded sigmoid_and_others set)
            for n in range(NCHUNKS):
                sl = slice(n * NCHUNK, (n + 1) * NCHUNK)
                for k in range(12, 15):
                    seng = nc.vector if (k + n) % 2 == 0 else nc.gpsimd
                    seng.tensor_mul(
                        sqtail[k][:, sl], utiles[k][:, sl], utiles[k][:, sl]
                    )
            for n in range(NCHUNKS):
                sl = slice(n * NCHUNK, (n + 1) * NCHUNK)
                nc.sync.dma_start(
                    uL[:, sl], ut[:, kL * BS + n * NCHUNK:kL * BS + (n + 1) * NCHUNK]
                )
                nc.tensor.matmul(
                    psumA[n][:, :], vw_L, uL[:, sl], start=False, stop=True,
                )
                seng = nc.gpsimd if n % 2 == 0 else nc.vector
                seng.tensor_mul(sqL[:, sl], uL[:, sl], uL[:, sl])
                # epilogue copy can start as soon as psumA[n] stops
                rhs = redrhs.tile([M, NCHUNK], F16, name=f"rhs{n}", tag="rhs")
                rhstiles[n] = rhs
                nc.scalar.activation(rhs[:, :], psumA[n][:, :], AF.Copy)
                meng = nc.vector if n % 2 == 0 else nc.gpsimd
                meng.tensor_mul(
                    rhs[0:EMBED, :], rhs[0:EMBED, :], rhs[0:EMBED, :]
                )
            nc.sync.dma_start(b_sb[:, :], bvec[:, :])
            for n in range(NCHUNKS):
                sl = slice(n * NCHUNK, (n + 1) * NCHUNK)
                for k in range(12, 15):
                    bmm(n, sqtail[k][:, sl])
                bmm(n, sqL[:, sl])
                nc.tensor.matmul(
                    psumB[n][:, :], red_sb[:, :], rhstiles[n][:, :],
                    start=False, stop=True,
                )
                out_sb = outp.tile([1, NCHUNK], F32, name=f"out{n}", tag="out")
                nc.scalar.activation(
                    out_sb[:, :], psumB[n][:, :], AF.Sigmoid, bias=b_sb[0:1, 0:1]
                )
                nc.sync.dma_start(y[n:n + 1, :], out_sb[:, :])

    nc.compile()
    return nc


_NC_CACHE = None


def _prep_inputs(x, w, b, v):
    x = np.asarray(x, dtype=np.float32)
    w = np.asarray(w, dtype=np.float32).reshape(FIELD)
    v = np.asarray(v, dtype=np.float32)
    b0 = float(np.asarray(b, dtype=np.float32).reshape(-1)[0])

    s64 = (v.astype(np.float64) ** 2).sum(axis=1)
    sqs = np.sqrt(s64)                      # [FIELD]
    vp = (v / sqs[:, None].astype(np.float32)).astype(np.float16)
    wp = (w / sqs.astype(np.float32)).astype(np.float16)
    vw = np.concatenate([vp, wp[:, None]], axis=1)  # [FIELD, M] fp16

    vwi = np.ascontiguousarray(
        vw.reshape(KTILES, 128, M).transpose(1, 0, 2).reshape(128, KTILES * M)
    )
    bvec = np.full((1, 1), b0, np.float32)

    u = (x * sqs.astype(np.float32)[None, :]).astype(np.float16)  # [BATCH, FIELD]

    in_maps = []
    for c in range(NCORES):
        uc = u[c * BS:(c + 1) * BS, :].T          # [FIELD, BS]
        ut_c = np.ascontiguousarray(
            uc.reshape(KTILES, 128, BS).transpose(1, 0, 2).reshape(128, KTILES * BS)
        )
        in_maps.append({"ut": ut_c, "vwi": vwi, "bvec": bvec})
    return in_maps


def _run(x, w, b, v, **spmd_kwargs):
    global _NC_CACHE
    if _NC_CACHE is None:
        _NC_CACHE = _build_nc()
    nc = _NC_CACHE

    in_maps = _prep_inputs(x, w, b, v)
    res = run_bass_kernel_spmd(nc, in_maps, list(range(NCORES)), **spmd_kwargs)
    out = np.concatenate(
        [res.results[c]["y"].reshape(BS) for c in range(NCORES)]
    )
    return out.reshape(BATCH, 1).astype(np.float32), res


def kernel(x, w, b, v):
    out, _ = _run(x, w, b, v)
    return out


# revision 35
# speedup vs baseline: 1.1234x; 1.1234x over previous
"""DeepFM forward kernel for 8 Trainium2 NeuronCores (Bass/Tile).

Math (per batch row b):
    lin[b] = x[b] @ w
    C[b]   = sum_k (x[b] @ v)_k^2
    B[b]   = sum_f s[f] * x[b,f]^2,   s[f] = sum_k v[f,k]^2
    out[b] = sigmoid(lin[b] + b0 + 0.5*C[b] - 0.5*B[b])

Data-parallel: batch 16384 sharded 8 ways (2048 rows/core); parameters
replicated.

Key reformulation: ship u = x * sqrt(s) (per-feature scale folded on host)
in fp16, with v' = v/sqrt(s), w' = w/sqrt(s) as the stationary matrix.
Then xv = u @ v', lin = u @ w', and B = sum_f u_f^2 — the only on-chip
elementwise op is an unscaled square. fp16 halves HBM traffic and runs
the PE at full rate.

Schedule notes (cost-model driven; ~29.3us/core vs 81.2us baseline):
  - u streams on BOTH HWDGE queues (SP: even stripes + quartered stripes
    0/15; ACT: odd stripes + 14) — transfers from different queues
    overlap in the DMA fabric, ~2x effective rate; stream ends ~14us.
  - B routing: stripes {0,12,13,14,15} feed ones-matmuls into PSUM on
    the PE; stripes 1..11 accumulate u^2 into four fp16 chain
    accumulators (DVE adds), combined pairwise and folded with two
    4-matmul sets mid-stream.
  - Squares split across DVE (1127ns) and GPS (1707ns); late stripes
    (10..15) split half/half so neither queue-end sticks out.
  - Constants materialize via DVE memsets; only scalar b rides a DMA.
  - ACT runs only Copy/Sigmoid; a warmup Sigmoid makes the single
    (hoisted) act-table load pick a set covering both, so no table load
    ever lands on the critical path.
  - Tail: all remaining A-matmuls (stripes 12-15) issue before any late
    B-matmul so psumA stops early; per chunk: copy [xv;lin]->fp16 (ACT),
    square-in-place (DVE/GPS), 4 B-matmuls + C-matmul (PE), Sigmoid,
    y DMA — chunk-pipelined.
"""

import numpy as np

import concourse.bass as bass
import concourse.tile as tile
from concourse import bacc, mybir
from concourse.bass_utils import run_bass_kernel_spmd

BATCH, FIELD, EMBED = 16384, 2048, 64
NCORES = 8
BS = BATCH // NCORES   # 2048 batch rows per core
NCHUNK = 512           # psum free-dim per matmul
KTILES = FIELD // 128  # 16 contraction stripes
NCHUNKS = BS // NCHUNK  # 4 batch chunks per core
M = EMBED + 1          # 65 stationary columns: v' plus w'

F32 = mybir.dt.float32
F16 = mybir.dt.float16
AF = mybir.ActivationFunctionType

# B-accumulation chains (value = chain id 0..3); seeds are the first
# member of each chain (its sq writes the accumulator directly).
CHAINS = [[1, 2, 3], [4, 5, 6], [7, 8, 9], [10, 11]]
PE_B = {0, 12, 13, 14, 15}
# Engine for each stripe's square: v=DVE, g=GPS(pool), s=split DVE+GPS
SQ_ENG = {1: "g", 2: "v", 3: "g", 4: "g", 5: "v", 6: "g", 7: "g",
          8: "g", 9: "v", 10: "s", 11: "s", 12: "s", 13: "s", 14: "s"}


def _build_nc():
    nc = bacc.Bacc("TRN2", target_bir_lowering=False, debug=False)

    # stripe-major u: partition p, col k*BS + b  <->  u[k*128+p, b]
    ut = nc.declare_dram_parameter("ut", [128, KTILES * BS], F16, isOutput=False)
    vwi = nc.declare_dram_parameter("vwi", [128, KTILES * M], F16, isOutput=False)
    bvec = nc.declare_dram_parameter("bvec", [1, 1], F32, isOutput=False)
    y = nc.declare_dram_parameter("y", [NCHUNKS, NCHUNK], F32, isOutput=True)

    with tile.TileContext(nc) as tc:
        with (
            tc.tile_pool(name="consts", bufs=1) as consts,
            tc.tile_pool(name="uin", bufs=8) as uin,
            tc.tile_pool(name="uq", bufs=1) as uq,
            tc.tile_pool(name="sqp", bufs=8) as sqp,
            tc.tile_pool(name="accs", bufs=1) as accs,
            tc.tile_pool(name="redrhs", bufs=4) as redrhs,
            tc.tile_pool(name="outp", bufs=2) as outp,
            tc.tile_pool(name="psA", bufs=NCHUNKS, space="PSUM") as psA,
            tc.tile_pool(name="psB", bufs=NCHUNKS, space="PSUM") as psB,
        ):
            vw = consts.tile([128, KTILES * M], F16)
            nc.scalar.dma_start(vw[:, :], vwi[:, :])
            b_sb = consts.tile([1, 1], F32)
            onesn_sb = consts.tile([128, 1], F16)
            nc.vector.memset(onesn_sb[:, :], -0.5)
            red_sb = consts.tile([M, 1], F16)
            nc.vector.memset(red_sb[0:EMBED, :], 0.5)
            nc.vector.memset(red_sb[EMBED:M, :], 1.0)

            psumA = [
                psA.tile([M, NCHUNK], F32, name=f"psumA{n}", tag="psumA")
                for n in range(NCHUNKS)
            ]
            psumB = [
                psB.tile([1, NCHUNK], F32, name=f"psumB{n}", tag="psumB")
                for n in range(NCHUNKS)
            ]

            acc = [accs.tile([128, BS], F16, name=f"acc{i}") for i in range(4)]
            chain_of = {k: ci for ci, ch in enumerate(CHAINS) for k in ch}
            seeds = {ch[0] for ch in CHAINS}

            utiles = {}

            ACT_STRIPES = {1, 3, 5, 7, 9, 11, 13, 14}

            def load(k):
                eng = nc.scalar if k in ACT_STRIPES else nc.sync
                t = uin.tile([128, BS], F16, name=f"u{k}", tag="u")
                eng.dma_start(t[:, :], ut[:, k * BS:(k + 1) * BS])
                utiles[k] = t

            first_b = [True] * NCHUNKS

            def bmm(n, src_cols):
                nc.tensor.matmul(
                    psumB[n][:, :], onesn_sb[:, :], src_cols,
                    start=first_b[n], stop=False,
                )
                first_b[n] = False

            def process(k):
                vw_k = vw[:, k * M:(k + 1) * M]
                u_k = utiles[k]
                if k in seeds:
                    sq_k = acc[chain_of[k]]
                else:
                    sq_k = sqp.tile([128, BS], F16, name=f"sq{k}", tag="sq")
                for n in range(NCHUNKS):
                    sl = slice(n * NCHUNK, (n + 1) * NCHUNK)
                    nc.tensor.matmul(
                        psumA[n][:, :], vw_k, u_k[:, sl],
                        start=False, stop=False,
                    )
                eng = SQ_ENG[k]
                if eng == "v":
                    nc.vector.tensor_mul(sq_k[:, :], u_k[:, :], u_k[:, :])
                elif eng == "g":
                    nc.gpsimd.tensor_mul(sq_k[:, :], u_k[:, :], u_k[:, :])
                else:
                    h = BS // 2
                    nc.vector.tensor_mul(sq_k[:, :h], u_k[:, :h], u_k[:, :h])
                    nc.gpsimd.tensor_mul(sq_k[:, h:], u_k[:, h:], u_k[:, h:])
                if k in PE_B:
                    for n in range(NCHUNKS):
                        sl = slice(n * NCHUNK, (n + 1) * NCHUNK)
                        bmm(n, sq_k[:, sl])
                elif k not in seeds:
                    a = acc[chain_of[k]]
                    nc.vector.tensor_add(a[:, :], a[:, :], sq_k[:, :])

            # stripe 0 on SP, quartered so the PE (and GPS/DVE) start early
            u0 = uq.tile([128, BS], F16, name="uqt0", tag="uq0")
            sq0 = sqp.tile([128, BS], F16, name="sq0", tag="sq")
            vw_0 = vw[:, 0:M]
            for n in range(NCHUNKS):
                sl = slice(n * NCHUNK, (n + 1) * NCHUNK)
                nc.sync.dma_start(u0[:, sl], ut[:, n * NCHUNK:(n + 1) * NCHUNK])
                nc.tensor.matmul(
                    psumA[n][:, :], vw_0, u0[:, sl], start=True, stop=False,
                )
                eng0 = nc.gpsimd
                eng0.tensor_mul(sq0[:, sl], u0[:, sl], u0[:, sl])
                bmm(n, sq0[:, sl])
            for k in range(1, KTILES - 1):
                load(k)
            warm = consts.tile([1, 1], F16)
            nc.scalar.activation(warm[:, :], onesn_sb[0:1, 0:1], AF.Sigmoid)

            for k in range(1, 7):
                process(k)
            # c1 += c2 (chunk-sliced so later folds stagger per chunk)
            for n in range(NCHUNKS):
                sl = slice(n * NCHUNK, (n + 1) * NCHUNK)
                nc.vector.tensor_add(acc[0][:, sl], acc[0][:, sl], acc[1][:, sl])
            for k in range(7, 10):
                process(k)
            # fold c12 as soon as it is complete
            for n in range(NCHUNKS):
                sl = slice(n * NCHUNK, (n + 1) * NCHUNK)
                bmm(n, acc[0][:, sl])
            process(10)
            process(11)
            # c3 += c4 on GPS (chunk-sliced), then fold it
            for n in range(NCHUNKS):
                sl = slice(n * NCHUNK, (n + 1) * NCHUNK)
                nc.vector.tensor_add(acc[2][:, sl], acc[2][:, sl], acc[3][:, sl])
            for n in range(NCHUNKS):
                sl = slice(n * NCHUNK, (n + 1) * NCHUNK)
                bmm(n, acc[2][:, sl])
            # ---- tail: A-matmuls first (unblock psumA stops), then late
            # B-matmuls, then the chunk-pipelined epilogue ----
            kL = KTILES - 1
            uL = uq.tile([128, BS], F16, name="uqt15", tag="uq15")
            sqL = sqp.tile([128, BS], F16, name="sq15", tag="sq")
            vw_L = vw[:, kL * M:(kL + 1) * M]
            sqtail = {}
            rhstiles = {}
            for k in range(12, 15):
                vw_k = vw[:, k * M:(k + 1) * M]
                u_k = utiles[k]
                for n in range(NCHUNKS):
                    sl = slice(n * NCHUNK, (n + 1) * NCHUNK)
                    nc.tensor.matmul(
                        psumA[n][:, :], vw_k, u_k[:, sl],
                        start=False, stop=False,
                    )
                sqtail[k] = sqp.tile([128, BS], F16, name=f"sq{k}", tag="sq")
            # chunk-sliced squares so each chunk's B-matmuls unblock as
            # soon as its own slices land; stripe 12 rides ACT's idle
            # window (Square is in the loaded sigmoid_and_others set)
            for n in range(NCHUNKS):
                sl = slice(n * NCHUNK, (n + 1) * NCHUNK)
                for k in range(12, 15):
                    seng = nc.vector if (k + n) % 2 == 0 else nc.gpsimd
                    seng.tensor_mul(
                        sqtail[k][:, sl], utiles[k][:, sl], utiles[k][:, sl]
                    )
            for n in range(NCHUNKS):
                sl = slice(n * NCHUNK, (n + 1) * NCHUNK)
                nc.sync.dma_start(
                    uL[:, sl], ut[:, kL * BS + n * NCHUNK:kL * BS + (n + 1) * NCHUNK]
                )
                nc.tensor.matmul(
                    psumA[n][:, :], vw_L, uL[:, sl], start=False, stop=True,
                )
                seng = nc.gpsimd if n % 2 == 0 else nc.vector
                seng.tensor_mul(sqL[:, sl], uL[:, sl], uL[:, sl])
                # epilogue copy can start as soon as psumA[n] stops
                rhs = redrhs.tile([M, NCHUNK], F16, name=f"rhs{n}", tag="rhs")
                rhstiles[n] = rhs
                nc.scalar.activation(rhs[:, :], psumA[n][:, :], AF.Copy)
                meng = nc.vector if n % 2 == 0 else nc.gpsimd
                meng.tensor_mul(
                    rhs[0:EMBED, :], rhs[0:EMBED, :], rhs[0:EMBED, :]
                )
            nc.sync.dma_start(b_sb[:, :], bvec[:, :])
            for n in range(NCHUNKS):
                sl = slice(n * NCHUNK, (n + 1) * NCHUNK)
                for k in range(12, 15):
                    bmm(n, sqtail[k][:, sl])
                bmm(n, sqL[:, sl])
                nc.tensor.matmul(
                    psumB[n][:, :], red_sb[:, :], rhstiles[n][:, :],
                    start=False, stop=True,
                )
                out_sb = outp.tile([1, NCHUNK], F32, name=f"out{n}", tag="out")
                nc.scalar.activation(
                    out_sb[:, :], psumB[n][:, :], AF.Sigmoid, bias=b_sb[0:1, 0:1]
                )
                nc.sync.dma_start(y[n:n + 1, :], out_sb[:, :])

    nc.compile()
    return nc


_NC_CACHE = None


def _prep_inputs(x, w, b, v):
    x = np.asarray(x, dtype=np.float32)
    w = np.asarray(w, dtype=np.float32).reshape(FIELD)
    v = np.asarray(v, dtype=np.float32)
    b0 = float(np.asarray(b, dtype=np.float32).reshape(-1)[0])

    s64 = (v.astype(np.float64) ** 2).sum(axis=1)
    sqs = np.sqrt(s64)                      # [FIELD]
    vp = (v / sqs[:, None].astype(np.float32)).astype(np.float16)
    wp = (w / sqs.astype(np.float32)).astype(np.float16)
    vw = np.concatenate([vp, wp[:, None]], axis=1)  # [FIELD, M] fp16

    vwi = np.ascontiguousarray(
        vw.reshape(KTILES, 128, M).transpose(1, 0, 2).reshape(128, KTILES * M)
    )
    bvec = np.full((1, 1), b0, np.float32)

    u = (x * sqs.astype(np.float32)[None, :]).astype(np.float16)  # [BATCH, FIELD]

    in_maps = []
    for c in range(NCORES):
        uc = u[c * BS:(c + 1) * BS, :].T          # [FIELD, BS]
        ut_c = np.ascontiguousarray(
            uc.reshape(KTILES, 128, BS).transpose(1, 0, 2).reshape(128, KTILES * BS)
        )
        in_maps.append({"ut": ut_c, "vwi": vwi, "bvec": bvec})
    return in_maps


def _run(x, w, b, v, **spmd_kwargs):
    global _NC_CACHE
    if _NC_CACHE is None:
        _NC_CACHE = _build_nc()
    nc = _NC_CACHE

    in_maps = _prep_inputs(x, w, b, v)
    res = run_bass_kernel_spmd(nc, in_maps, list(range(NCORES)), **spmd_kwargs)
    out = np.concatenate(
        [res.results[c]["y"].reshape(BS) for c in range(NCORES)]
    )
    return out.reshape(BATCH, 1).astype(np.float32), res


def kernel(x, w, b, v):
    out, _ = _run(x, w, b, v)
    return out


# revision 36
# speedup vs baseline: 1.1366x; 1.0118x over previous
"""DeepFM forward kernel for 8 Trainium2 NeuronCores (Bass/Tile).

Math (per batch row b):
    lin[b] = x[b] @ w
    C[b]   = sum_k (x[b] @ v)_k^2
    B[b]   = sum_f s[f] * x[b,f]^2,   s[f] = sum_k v[f,k]^2
    out[b] = sigmoid(lin[b] + b0 + 0.5*C[b] - 0.5*B[b])

Data-parallel: batch 16384 sharded 8 ways (2048 rows/core); parameters
replicated.

Key reformulation: ship u = x * sqrt(s) (per-feature scale folded on host)
in fp16, with v' = v/sqrt(s), w' = w/sqrt(s) as the stationary matrix.
Then xv = u @ v', lin = u @ w', and B = sum_f u_f^2 — the only on-chip
elementwise op is an unscaled square. fp16 halves HBM traffic and runs
the PE at full rate.

Schedule notes (cost-model driven; ~29.3us/core vs 81.2us baseline):
  - u streams on BOTH HWDGE queues (SP: even stripes + quartered stripes
    0/15; ACT: odd stripes + 14) — transfers from different queues
    overlap in the DMA fabric, ~2x effective rate; stream ends ~14us.
  - B routing: stripes {0,12,13,14,15} feed ones-matmuls into PSUM on
    the PE; stripes 1..11 accumulate u^2 into four fp16 chain
    accumulators (DVE adds), combined pairwise and folded with two
    4-matmul sets mid-stream.
  - Squares split across DVE (1127ns) and GPS (1707ns); late stripes
    (10..15) split half/half so neither queue-end sticks out.
  - Constants materialize via DVE memsets; only scalar b rides a DMA.
  - ACT runs only Copy/Sigmoid; a warmup Sigmoid makes the single
    (hoisted) act-table load pick a set covering both, so no table load
    ever lands on the critical path.
  - Tail: all remaining A-matmuls (stripes 12-15) issue before any late
    B-matmul so psumA stops early; per chunk: copy [xv;lin]->fp16 (ACT),
    square-in-place (DVE/GPS), 4 B-matmuls + C-matmul (PE), Sigmoid,
    y DMA — chunk-pipelined.
"""

import numpy as np

import concourse.bass as bass
import concourse.tile as tile
from concourse import bacc, mybir
from concourse.bass_utils import run_bass_kernel_spmd

BATCH, FIELD, EMBED = 16384, 2048, 64
NCORES = 8
BS = BATCH // NCORES   # 2048 batch rows per core
NCHUNK = 512           # psum free-dim per matmul
KTILES = FIELD // 128  # 16 contraction stripes
NCHUNKS = BS // NCHUNK  # 4 batch chunks per core
M = EMBED + 1          # 65 stationary columns: v' plus w'

F32 = mybir.dt.float32
F16 = mybir.dt.float16
AF = mybir.ActivationFunctionType

# B-accumulation chains (value = chain id 0..3); seeds are the first
# member of each chain (its sq writes the accumulator directly).
CHAINS = [[1, 2, 3], [4, 5, 6], [7, 8, 9], [10, 11]]
PE_B = {0, 12, 13, 14, 15}
# Engine for each stripe's square: v=DVE, g=GPS(pool), s=split DVE+GPS
SQ_ENG = {1: "g", 2: "v", 3: "g", 4: "g", 5: "v", 6: "g", 7: "g",
          8: "g", 9: "v", 10: "s", 11: "s", 12: "s", 13: "s", 14: "s"}


def _build_nc():
    nc = bacc.Bacc("TRN2", target_bir_lowering=False, debug=False)

    # stripe-major u: partition p, col k*BS + b  <->  u[k*128+p, b]
    ut = nc.declare_dram_parameter("ut", [128, KTILES * BS], F16, isOutput=False)
    vwi = nc.declare_dram_parameter("vwi", [128, KTILES * M], F16, isOutput=False)
    bvec = nc.declare_dram_parameter("bvec", [1, 1], F32, isOutput=False)
    y = nc.declare_dram_parameter("y", [NCHUNKS, NCHUNK], F32, isOutput=True)

    with tile.TileContext(nc) as tc:
        with (
            tc.tile_pool(name="consts", bufs=1) as consts,
            tc.tile_pool(name="uin", bufs=8) as uin,
            tc.tile_pool(name="uq", bufs=1) as uq,
            tc.tile_pool(name="sqp", bufs=8) as sqp,
            tc.tile_pool(name="accs", bufs=1) as accs,
            tc.tile_pool(name="redrhs", bufs=4) as redrhs,
            tc.tile_pool(name="outp", bufs=2) as outp,
            tc.tile_pool(name="psA", bufs=NCHUNKS, space="PSUM") as psA,
            tc.tile_pool(name="psB", bufs=NCHUNKS, space="PSUM") as psB,
        ):
            vw = consts.tile([128, KTILES * M], F16)
            nc.scalar.dma_start(vw[:, :], vwi[:, :])
            b_sb = consts.tile([1, 1], F32)
            onesn_sb = consts.tile([128, 1], F16)
            nc.vector.memset(onesn_sb[:, :], -0.5)
            red_sb = consts.tile([M, 1], F16)
            nc.vector.memset(red_sb[0:EMBED, :], 0.5)
            nc.vector.memset(red_sb[EMBED:M, :], 1.0)

            psumA = [
                psA.tile([M, NCHUNK], F32, name=f"psumA{n}", tag="psumA")
                for n in range(NCHUNKS)
            ]
            psumB = [
                psB.tile([1, NCHUNK], F32, name=f"psumB{n}", tag="psumB")
                for n in range(NCHUNKS)
            ]

            acc = [accs.tile([128, BS], F16, name=f"acc{i}") for i in range(4)]
            chain_of = {k: ci for ci, ch in enumerate(CHAINS) for k in ch}
            seeds = {ch[0] for ch in CHAINS}

            utiles = {}

            ACT_STRIPES = {1, 3, 5, 7, 9, 11, 13, 14}

            def load(k):
                eng = nc.scalar if k in ACT_STRIPES else nc.sync
                t = uin.tile([128, BS], F16, name=f"u{k}", tag="u")
                eng.dma_start(t[:, :], ut[:, k * BS:(k + 1) * BS])
                utiles[k] = t

            first_b = [True] * NCHUNKS

            def bmm(n, src_cols):
                nc.tensor.matmul(
                    psumB[n][:, :], onesn_sb[:, :], src_cols,
                    start=first_b[n], stop=False,
                )
                first_b[n] = False

            def process(k):
                vw_k = vw[:, k * M:(k + 1) * M]
                u_k = utiles[k]
                if k in seeds:
                    sq_k = acc[chain_of[k]]
                else:
                    sq_k = sqp.tile([128, BS], F16, name=f"sq{k}", tag="sq")
                for n in range(NCHUNKS):
                    sl = slice(n * NCHUNK, (n + 1) * NCHUNK)
                    nc.tensor.matmul(
                        psumA[n][:, :], vw_k, u_k[:, sl],
                        start=False, stop=False,
                    )
                eng = SQ_ENG[k]
                if eng == "v":
                    nc.vector.tensor_mul(sq_k[:, :], u_k[:, :], u_k[:, :])
                elif eng == "g":
                    nc.gpsimd.tensor_mul(sq_k[:, :], u_k[:, :], u_k[:, :])
                else:
                    h = BS // 2
                    nc.vector.tensor_mul(sq_k[:, :h], u_k[:, :h], u_k[:, :h])
                    nc.gpsimd.tensor_mul(sq_k[:, h:], u_k[:, h:], u_k[:, h:])
                if k in PE_B:
                    for n in range(NCHUNKS):
                        sl = slice(n * NCHUNK, (n + 1) * NCHUNK)
                        bmm(n, sq_k[:, sl])
                elif k not in seeds:
                    a = acc[chain_of[k]]
                    nc.vector.tensor_add(a[:, :], a[:, :], sq_k[:, :])

            # stripe 0 on SP, quartered so the PE (and GPS/DVE) start early
            u0 = uq.tile([128, BS], F16, name="uqt0", tag="uq0")
            sq0 = sqp.tile([128, BS], F16, name="sq0", tag="sq")
            vw_0 = vw[:, 0:M]
            for n in range(NCHUNKS):
                sl = slice(n * NCHUNK, (n + 1) * NCHUNK)
                nc.sync.dma_start(u0[:, sl], ut[:, n * NCHUNK:(n + 1) * NCHUNK])
                nc.tensor.matmul(
                    psumA[n][:, :], vw_0, u0[:, sl], start=True, stop=False,
                )
                eng0 = nc.gpsimd if n < 2 else nc.vector
                eng0.tensor_mul(sq0[:, sl], u0[:, sl], u0[:, sl])
                bmm(n, sq0[:, sl])
            for k in range(1, KTILES - 1):
                load(k)
            warm = consts.tile([1, 1], F16)
            nc.scalar.activation(warm[:, :], onesn_sb[0:1, 0:1], AF.Sigmoid)

            for k in range(1, 7):
                process(k)
            # c1 += c2 (chunk-sliced so later folds stagger per chunk)
            for n in range(NCHUNKS):
                sl = slice(n * NCHUNK, (n + 1) * NCHUNK)
                nc.vector.tensor_add(acc[0][:, sl], acc[0][:, sl], acc[1][:, sl])
            for k in range(7, 10):
                process(k)
            # fold c12 as soon as it is complete
            for n in range(NCHUNKS):
                sl = slice(n * NCHUNK, (n + 1) * NCHUNK)
                bmm(n, acc[0][:, sl])
            process(10)
            process(11)
            # c3 += c4 on GPS (chunk-sliced), then fold it
            for n in range(NCHUNKS):
                sl = slice(n * NCHUNK, (n + 1) * NCHUNK)
                nc.vector.tensor_add(acc[2][:, sl], acc[2][:, sl], acc[3][:, sl])
            for n in range(NCHUNKS):
                sl = slice(n * NCHUNK, (n + 1) * NCHUNK)
                bmm(n, acc[2][:, sl])
            # ---- tail: A-matmuls first (unblock psumA stops), then late
            # B-matmuls, then the chunk-pipelined epilogue ----
            kL = KTILES - 1
            uL = uq.tile([128, BS], F16, name="uqt15", tag="uq15")
            sqL = sqp.tile([128, BS], F16, name="sq15", tag="sq")
            vw_L = vw[:, kL * M:(kL + 1) * M]
            sqtail = {}
            rhstiles = {}
            for k in range(12, 15):
                vw_k = vw[:, k * M:(k + 1) * M]
                u_k = utiles[k]
                for n in range(NCHUNKS):
                    sl = slice(n * NCHUNK, (n + 1) * NCHUNK)
                    nc.tensor.matmul(
                        psumA[n][:, :], vw_k, u_k[:, sl],
                        start=False, stop=False,
                    )
                sqtail[k] = sqp.tile([128, BS], F16, name=f"sq{k}", tag="sq")
            # chunk-sliced squares so each chunk's B-matmuls unblock as
            # soon as its own slices land; stripe 12 rides ACT's idle
            # window (Square is in the loaded sigmoid_and_others set)
            for n in range(NCHUNKS):
                sl = slice(n * NCHUNK, (n + 1) * NCHUNK)
                for k in range(12, 15):
                    seng = nc.vector if (k + n) % 2 == 0 else nc.gpsimd
                    seng.tensor_mul(
                        sqtail[k][:, sl], utiles[k][:, sl], utiles[k][:, sl]
                    )
            for n in range(NCHUNKS):
                sl = slice(n * NCHUNK, (n + 1) * NCHUNK)
                nc.sync.dma_start(
                    uL[:, sl], ut[:, kL * BS + n * NCHUNK:kL * BS + (n + 1) * NCHUNK]
                )
                nc.tensor.matmul(
                    psumA[n][:, :], vw_L, uL[:, sl], start=False, stop=True,
                )
                seng = nc.gpsimd if n % 2 == 0 else nc.vector
                seng.tensor_mul(sqL[:, sl], uL[:, sl], uL[:, sl])
                # epilogue copy can start as soon as psumA[n] stops
                rhs = redrhs.tile([M, NCHUNK], F16, name=f"rhs{n}", tag="rhs")
                rhstiles[n] = rhs
                nc.scalar.activation(rhs[:, :], psumA[n][:, :], AF.Copy)
                nc.gpsimd.tensor_mul(
                    rhs[0:EMBED, :], rhs[0:EMBED, :], rhs[0:EMBED, :]
                )
            nc.sync.dma_start(b_sb[:, :], bvec[:, :])
            for n in range(NCHUNKS):
                sl = slice(n * NCHUNK, (n + 1) * NCHUNK)
                for k in range(12, 15):
                    bmm(n, sqtail[k][:, sl])
                bmm(n, sqL[:, sl])
                nc.tensor.matmul(
                    psumB[n][:, :], red_sb[:, :], rhstiles[n][:, :],
                    start=False, stop=True,
                )
                out_sb = outp.tile([1, NCHUNK], F32, name=f"out{n}", tag="out")
                nc.scalar.activation(
                    out_sb[:, :], psumB[n][:, :], AF.Sigmoid, bias=b_sb[0:1, 0:1]
                )
                nc.sync.dma_start(y[n:n + 1, :], out_sb[:, :])

    nc.compile()
    return nc


_NC_CACHE = None


def _prep_inputs(x, w, b, v):
    x = np.asarray(x, dtype=np.float32)
    w = np.asarray(w, dtype=np.float32).reshape(FIELD)
    v = np.asarray(v, dtype=np.float32)
    b0 = float(np.asarray(b, dtype=np.float32).reshape(-1)[0])

    s64 = (v.astype(np.float64) ** 2).sum(axis=1)
    sqs = np.sqrt(s64)                      # [FIELD]
    vp = (v / sqs[:, None].astype(np.float32)).astype(np.float16)
    wp = (w / sqs.astype(np.float32)).astype(np.float16)
    vw = np.concatenate([vp, wp[:, None]], axis=1)  # [FIELD, M] fp16

    vwi = np.ascontiguousarray(
        vw.reshape(KTILES, 128, M).transpose(1, 0, 2).reshape(128, KTILES * M)
    )
    bvec = np.full((1, 1), b0, np.float32)

    u = (x * sqs.astype(np.float32)[None, :]).astype(np.float16)  # [BATCH, FIELD]

    in_maps = []
    for c in range(NCORES):
        uc = u[c * BS:(c + 1) * BS, :].T          # [FIELD, BS]
        ut_c = np.ascontiguousarray(
            uc.reshape(KTILES, 128, BS).transpose(1, 0, 2).reshape(128, KTILES * BS)
        )
        in_maps.append({"ut": ut_c, "vwi": vwi, "bvec": bvec})
    return in_maps


def _run(x, w, b, v, **spmd_kwargs):
    global _NC_CACHE
    if _NC_CACHE is None:
        _NC_CACHE = _build_nc()
    nc = _NC_CACHE

    in_maps = _prep_inputs(x, w, b, v)
    res = run_bass_kernel_spmd(nc, in_maps, list(range(NCORES)), **spmd_kwargs)
    out = np.concatenate(
        [res.results[c]["y"].reshape(BS) for c in range(NCORES)]
    )
    return out.reshape(BATCH, 1).astype(np.float32), res


def kernel(x, w, b, v):
    out, _ = _run(x, w, b, v)
    return out


# revision 37
# speedup vs baseline: 1.1620x; 1.0223x over previous
"""DeepFM forward kernel for 8 Trainium2 NeuronCores (Bass/Tile).

Math (per batch row b):
    lin[b] = x[b] @ w
    C[b]   = sum_k (x[b] @ v)_k^2
    B[b]   = sum_f s[f] * x[b,f]^2,   s[f] = sum_k v[f,k]^2
    out[b] = sigmoid(lin[b] + b0 + 0.5*C[b] - 0.5*B[b])

Data-parallel: batch 16384 sharded 8 ways (2048 rows/core); parameters
replicated.

Key reformulation: ship u = x * sqrt(s) (per-feature scale folded on host)
in fp16, with v' = v/sqrt(s), w' = w/sqrt(s) as the stationary matrix.
Then xv = u @ v', lin = u @ w', and B = sum_f u_f^2 — the only on-chip
elementwise op is an unscaled square. fp16 halves HBM traffic and runs
the PE at full rate.

Schedule notes (cost-model driven; ~29.3us/core vs 81.2us baseline):
  - u streams on BOTH HWDGE queues (SP: even stripes + quartered stripes
    0/15; ACT: odd stripes + 14) — transfers from different queues
    overlap in the DMA fabric, ~2x effective rate; stream ends ~14us.
  - B routing: stripes {0,12,13,14,15} feed ones-matmuls into PSUM on
    the PE; stripes 1..11 accumulate u^2 into four fp16 chain
    accumulators (DVE adds), combined pairwise and folded with two
    4-matmul sets mid-stream.
  - Squares split across DVE (1127ns) and GPS (1707ns); late stripes
    (10..15) split half/half so neither queue-end sticks out.
  - Constants materialize via DVE memsets; only scalar b rides a DMA.
  - ACT runs only Copy/Sigmoid; a warmup Sigmoid makes the single
    (hoisted) act-table load pick a set covering both, so no table load
    ever lands on the critical path.
  - Tail: all remaining A-matmuls (stripes 12-15) issue before any late
    B-matmul so psumA stops early; per chunk: copy [xv;lin]->fp16 (ACT),
    square-in-place (DVE/GPS), 4 B-matmuls + C-matmul (PE), Sigmoid,
    y DMA — chunk-pipelined.
"""

import numpy as np

import concourse.bass as bass
import concourse.tile as tile
from concourse import bacc, mybir
from concourse.bass_utils import run_bass_kernel_spmd

BATCH, FIELD, EMBED = 16384, 2048, 64
NCORES = 8
BS = BATCH // NCORES   # 2048 batch rows per core
NCHUNK = 512           # psum free-dim per matmul
KTILES = FIELD // 128  # 16 contraction stripes
NCHUNKS = BS // NCHUNK  # 4 batch chunks per core
M = EMBED + 1          # 65 stationary columns: v' plus w'

F32 = mybir.dt.float32
F16 = mybir.dt.float16
AF = mybir.ActivationFunctionType

# B-accumulation chains (value = chain id 0..3); seeds are the first
# member of each chain (its sq writes the accumulator directly).
CHAINS = [[1, 2, 3], [4, 5, 6], [7, 8, 9], [10, 11]]
PE_B = {0, 12, 13, 14, 15}
# Engine for each stripe's square: v=DVE, g=GPS(pool), s=split DVE+GPS
SQ_ENG = {1: "g", 2: "v", 3: "g", 4: "g", 5: "v", 6: "g", 7: "g",
          8: "g", 9: "v", 10: "s", 11: "s", 12: "s", 13: "s", 14: "s"}


def _build_nc():
    nc = bacc.Bacc("TRN2", target_bir_lowering=False, debug=False)

    # stripe-major u: partition p, col k*BS + b  <->  u[k*128+p, b]
    ut = nc.declare_dram_parameter("ut", [128, KTILES * BS], F16, isOutput=False)
    vwi = nc.declare_dram_parameter("vwi", [128, KTILES * M], F16, isOutput=False)
    bvec = nc.declare_dram_parameter("bvec", [1, 1], F32, isOutput=False)
    y = nc.declare_dram_parameter("y", [NCHUNKS, NCHUNK], F32, isOutput=True)

    with tile.TileContext(nc) as tc:
        with (
            tc.tile_pool(name="consts", bufs=1) as consts,
            tc.tile_pool(name="uin", bufs=8) as uin,
            tc.tile_pool(name="uq", bufs=1) as uq,
            tc.tile_pool(name="sqp", bufs=8) as sqp,
            tc.tile_pool(name="accs", bufs=1) as accs,
            tc.tile_pool(name="redrhs", bufs=4) as redrhs,
            tc.tile_pool(name="outp", bufs=2) as outp,
            tc.tile_pool(name="psA", bufs=NCHUNKS, space="PSUM") as psA,
            tc.tile_pool(name="psB", bufs=NCHUNKS, space="PSUM") as psB,
        ):
            vw = consts.tile([128, KTILES * M], F16)
            nc.scalar.dma_start(vw[:, :], vwi[:, :])
            b_sb = consts.tile([1, 1], F32)
            onesn_sb = consts.tile([128, 1], F16)
            nc.vector.memset(onesn_sb[:, :], -0.5)
            red_sb = consts.tile([M, 1], F16)
            nc.vector.memset(red_sb[0:EMBED, :], 0.5)
            nc.vector.memset(red_sb[EMBED:M, :], 1.0)

            psumA = [
                psA.tile([M, NCHUNK], F32, name=f"psumA{n}", tag="psumA")
                for n in range(NCHUNKS)
            ]
            psumB = [
                psB.tile([1, NCHUNK], F32, name=f"psumB{n}", tag="psumB")
                for n in range(NCHUNKS)
            ]

            acc = [accs.tile([128, BS], F16, name=f"acc{i}") for i in range(4)]
            chain_of = {k: ci for ci, ch in enumerate(CHAINS) for k in ch}
            seeds = {ch[0] for ch in CHAINS}

            utiles = {}

            ACT_STRIPES = {1, 3, 5, 7, 9, 11, 13, 14}

            def load(k):
                eng = nc.scalar if k in ACT_STRIPES else nc.sync
                t = uin.tile([128, BS], F16, name=f"u{k}", tag="u")
                eng.dma_start(t[:, :], ut[:, k * BS:(k + 1) * BS])
                utiles[k] = t

            first_b = [True] * NCHUNKS

            def bmm(n, src_cols):
                nc.tensor.matmul(
                    psumB[n][:, :], onesn_sb[:, :], src_cols,
                    start=first_b[n], stop=False,
                )
                first_b[n] = False

            def process(k):
                vw_k = vw[:, k * M:(k + 1) * M]
                u_k = utiles[k]
                if k in seeds:
                    sq_k = acc[chain_of[k]]
                else:
                    sq_k = sqp.tile([128, BS], F16, name=f"sq{k}", tag="sq")
                for n in range(NCHUNKS):
                    sl = slice(n * NCHUNK, (n + 1) * NCHUNK)
                    nc.tensor.matmul(
                        psumA[n][:, :], vw_k, u_k[:, sl],
                        start=False, stop=False,
                    )
                eng = SQ_ENG[k]
                if eng == "v":
                    nc.vector.tensor_mul(sq_k[:, :], u_k[:, :], u_k[:, :])
                elif eng == "g":
                    nc.gpsimd.tensor_mul(sq_k[:, :], u_k[:, :], u_k[:, :])
                else:
                    h = BS // 2
                    nc.vector.tensor_mul(sq_k[:, :h], u_k[:, :h], u_k[:, :h])
                    nc.gpsimd.tensor_mul(sq_k[:, h:], u_k[:, h:], u_k[:, h:])
                if k in PE_B:
                    for n in range(NCHUNKS):
                        sl = slice(n * NCHUNK, (n + 1) * NCHUNK)
                        bmm(n, sq_k[:, sl])
                elif k not in seeds:
                    a = acc[chain_of[k]]
                    nc.vector.tensor_add(a[:, :], a[:, :], sq_k[:, :])

            # stripe 0 on SP, quartered so the PE (and GPS/DVE) start early
            u0 = uq.tile([128, BS], F16, name="uqt0", tag="uq0")
            sq0 = sqp.tile([128, BS], F16, name="sq0", tag="sq")
            vw_0 = vw[:, 0:M]
            for n in range(NCHUNKS):
                sl = slice(n * NCHUNK, (n + 1) * NCHUNK)
                nc.sync.dma_start(u0[:, sl], ut[:, n * NCHUNK:(n + 1) * NCHUNK])
                nc.tensor.matmul(
                    psumA[n][:, :], vw_0, u0[:, sl], start=True, stop=False,
                )
                eng0 = nc.gpsimd if n < 2 else nc.vector
                eng0.tensor_mul(sq0[:, sl], u0[:, sl], u0[:, sl])
                bmm(n, sq0[:, sl])
            for k in range(1, KTILES - 1):
                load(k)
            warm = consts.tile([1, 1], F16)
            nc.scalar.activation(warm[:, :], onesn_sb[0:1, 0:1], AF.Sigmoid)

            for k in range(1, 7):
                process(k)
            for k in range(7, 10):
                process(k)
            # fold c1 and c2 directly (no combine: PE absorbs the extra
            # matmuls in its mid-stream idle; DVE sheds the slice-adds)
            for a in (acc[0], acc[1]):
                for n in range(NCHUNKS):
                    sl = slice(n * NCHUNK, (n + 1) * NCHUNK)
                    bmm(n, a[:, sl])
            process(10)
            process(11)
            for a in (acc[2], acc[3]):
                for n in range(NCHUNKS):
                    sl = slice(n * NCHUNK, (n + 1) * NCHUNK)
                    bmm(n, a[:, sl])
            # ---- tail: A-matmuls first (unblock psumA stops), then late
            # B-matmuls, then the chunk-pipelined epilogue ----
            kL = KTILES - 1
            uL = uq.tile([128, BS], F16, name="uqt15", tag="uq15")
            sqL = sqp.tile([128, BS], F16, name="sq15", tag="sq")
            vw_L = vw[:, kL * M:(kL + 1) * M]
            sqtail = {}
            rhstiles = {}
            for k in range(12, 15):
                vw_k = vw[:, k * M:(k + 1) * M]
                u_k = utiles[k]
                for n in range(NCHUNKS):
                    sl = slice(n * NCHUNK, (n + 1) * NCHUNK)
                    nc.tensor.matmul(
                        psumA[n][:, :], vw_k, u_k[:, sl],
                        start=False, stop=False,
                    )
                sqtail[k] = sqp.tile([128, BS], F16, name=f"sq{k}", tag="sq")
            # chunk-sliced squares so each chunk's B-matmuls unblock as
            # soon as its own slices land; stripe 12 rides ACT's idle
            # window (Square is in the loaded sigmoid_and_others set)
            for n in range(NCHUNKS):
                sl = slice(n * NCHUNK, (n + 1) * NCHUNK)
                for k in range(12, 15):
                    seng = nc.vector if (k + n) % 2 == 0 else nc.gpsimd
                    seng.tensor_mul(
                        sqtail[k][:, sl], utiles[k][:, sl], utiles[k][:, sl]
                    )
            for n in range(NCHUNKS):
                sl = slice(n * NCHUNK, (n + 1) * NCHUNK)
                nc.sync.dma_start(
                    uL[:, sl], ut[:, kL * BS + n * NCHUNK:kL * BS + (n + 1) * NCHUNK]
                )
                nc.tensor.matmul(
                    psumA[n][:, :], vw_L, uL[:, sl], start=False, stop=True,
                )
                seng = nc.gpsimd if n % 2 == 0 else nc.vector
                seng.tensor_mul(sqL[:, sl], uL[:, sl], uL[:, sl])
                # epilogue copy can start as soon as psumA[n] stops
                rhs = redrhs.tile([M, NCHUNK], F16, name=f"rhs{n}", tag="rhs")
                rhstiles[n] = rhs
                nc.scalar.activation(rhs[:, :], psumA[n][:, :], AF.Copy)
                meng = nc.vector if n % 2 == 0 else nc.gpsimd
                meng.tensor_mul(
                    rhs[0:EMBED, :], rhs[0:EMBED, :], rhs[0:EMBED, :]
                )
            nc.sync.dma_start(b_sb[:, :], bvec[:, :])
            for n in range(NCHUNKS):
                sl = slice(n * NCHUNK, (n + 1) * NCHUNK)
                for k in range(12, 15):
                    bmm(n, sqtail[k][:, sl])
                bmm(n, sqL[:, sl])
                nc.tensor.matmul(
                    psumB[n][:, :], red_sb[:, :], rhstiles[n][:, :],
                    start=False, stop=True,
                )
                out_sb = outp.tile([1, NCHUNK], F32, name=f"out{n}", tag="out")
                nc.scalar.activation(
                    out_sb[:, :], psumB[n][:, :], AF.Sigmoid, bias=b_sb[0:1, 0:1]
                )
                nc.sync.dma_start(y[n:n + 1, :], out_sb[:, :])

    nc.compile()
    return nc


_NC_CACHE = None


def _prep_inputs(x, w, b, v):
    x = np.asarray(x, dtype=np.float32)
    w = np.asarray(w, dtype=np.float32).reshape(FIELD)
    v = np.asarray(v, dtype=np.float32)
    b0 = float(np.asarray(b, dtype=np.float32).reshape(-1)[0])

    s64 = (v.astype(np.float64) ** 2).sum(axis=1)
    sqs = np.sqrt(s64)                      # [FIELD]
    vp = (v / sqs[:, None].astype(np.float32)).astype(np.float16)
    wp = (w / sqs.astype(np.float32)).astype(np.float16)
    vw = np.concatenate([vp, wp[:, None]], axis=1)  # [FIELD, M] fp16

    vwi = np.ascontiguousarray(
        vw.reshape(KTILES, 128, M).transpose(1, 0, 2).reshape(128, KTILES * M)
    )
    bvec = np.full((1, 1), b0, np.float32)

    u = (x * sqs.astype(np.float32)[None, :]).astype(np.float16)  # [BATCH, FIELD]

    in_maps = []
    for c in range(NCORES):
        uc = u[c * BS:(c + 1) * BS, :].T          # [FIELD, BS]
        ut_c = np.ascontiguousarray(
            uc.reshape(KTILES, 128, BS).transpose(1, 0, 2).reshape(128, KTILES * BS)
        )
        in_maps.append({"ut": ut_c, "vwi": vwi, "bvec": bvec})
    return in_maps


def _run(x, w, b, v, **spmd_kwargs):
    global _NC_CACHE
    if _NC_CACHE is None:
        _NC_CACHE = _build_nc()
    nc = _NC_CACHE

    in_maps = _prep_inputs(x, w, b, v)
    res = run_bass_kernel_spmd(nc, in_maps, list(range(NCORES)), **spmd_kwargs)
    out = np.concatenate(
        [res.results[c]["y"].reshape(BS) for c in range(NCORES)]
    )
    return out.reshape(BATCH, 1).astype(np.float32), res


def kernel(x, w, b, v):
    out, _ = _run(x, w, b, v)
    return out
